# revision 37
# baseline (speedup 1.0000x reference)
"""Multi-head attention (B=1, S=4096, D=768, H=12) on 8 trn2 NeuronCores.

Sharding: data-parallel over query rows (512 q rows/core) for attention;
the K/V projections are sharded over sequence (512 k rows/core) and
exchanged with a single HBM AllGather so no core does redundant
projection work. Per core:

  - projects its 512-row v shard (with the ones column appended -> the
    probs@V matmul also produces the softmax denominator) and its k
    shard (transposed layout), packs both into one bounce buffer,
    AllGathers, and DMAs the gathered slices back to SBUF contiguously
  - projects its q shard to QT [768, 512] while the AllGather runs
  - scores sT = K_h @ Q_h^T per head/k-tile on PE; exp directly from
    PSUM on ACT (scale=1/sqrt(64) folded into the activation affine);
    mask applied multiplicatively post-exp ({0,1} fp16) on DVE 2x mode
    (two groups per head offloaded to GPSIMD); probs@V accumulated on
    PE; per-head normalization via reciprocal_approx_fast
  - output projection to outT [768, 512]; host transposes/concats.

Inputs are pre-transposed/pre-tiled/cast to fp16 on the host (layout
prep is part of sharding); all matmul accumulation is fp32 in PSUM.
"""

import numpy as np

import concourse.bass as bass
import concourse.mybir as mybir
import concourse.tile as tile
from concourse import bacc, bass_utils

B, S, D, H = 1, 4096, 768, 12
DK = D // H  # 64
NCORES = 8
SQ = S // NCORES  # 512 query rows per core
KT_TILES = S // 128  # 32 k tiles
DT = D // 128  # 6 tiles of the model dim
KSH = S // NCORES  # 512 k rows per core (projection shard)

F16 = mybir.dt.float16
F32 = mybir.dt.float32

KPART = DT * KSH  # 3072 fp16 per partition: K shard in the exchange
VPART = 4 * H * (DK + 1)  # 3120: V shard (incl. ones column)

# exp group sizes per head: 3s so PSUM fits two score slots (2x3 banks)
# AND a double-buffered ctx accumulator (2x1 bank) — ctx x2 lets head h+1's
# probs@V start while head h's normalize chain drains.
EXP_GROUPS = [3, 3, 3, 3, 3, 3, 3, 3, 3, 3, 2]
assert sum(EXP_GROUPS) == KT_TILES
GPSIMD_GROUPS = ()  # mask-multiply groups offloaded to GPSIMD (hurt: FIFO HOL)

_CACHE = {}


def build_kernel(variant="full", timing=False):
    nc = bacc.Bacc("TRN2", target_bir_lowering=False, debug=False, num_devices=NCORES)

    # timing=True: declare the big inputs as Internal DRAM (garbage contents,
    # identical DMA traffic) so per-exec host->device input copies don't
    # dominate the wall-clock slope measurement.
    kw = {} if timing else {"kind": "ExternalInput"}
    qT = nc.dram_tensor("qT", [128, DT, SQ], F16, **kw)
    kTl = nc.dram_tensor("kTl", [128, DT, KSH], F16, **kw)
    vTl = nc.dram_tensor("vTl", [128, DT, KSH], F16, **kw)
    maskT = nc.dram_tensor("maskT", [128, KT_TILES, SQ], F16, **kw)
    w = {x: nc.dram_tensor(f"w{x}", [128, DT, D], F16, **kw) for x in "qkvo"}
    b = {x: nc.dram_tensor(f"b{x}", [128, DT], F32, **kw) for x in "qkvo"}
    b["v_rep"] = nc.dram_tensor("bv_rep", [128, H, DK], F32, **kw)
    if timing:
        nc.dram_tensor("tinput", [1, 8], F32, kind="ExternalInput")
    outT = nc.dram_tensor("outT", [D, SQ], F32, kind="ExternalOutput")
    dbg = {}
    if variant == "debug":
        dbg["kt"] = nc.dram_tensor("dbg_kt", [128, DT * S], F16, kind="ExternalOutput")
        dbg["v"] = nc.dram_tensor(
            "dbg_v", [128, KT_TILES * H * (DK + 1)], F16, kind="ExternalOutput"
        )
        dbg["q"] = nc.dram_tensor("dbg_q", [128, DT * SQ], F16, kind="ExternalOutput")
        dbg["ctx"] = nc.dram_tensor("dbg_ctx", [128, DT * SQ], F16, kind="ExternalOutput")
        dbg["den"] = nc.dram_tensor("dbg_den", [H, SQ], F32, kind="ExternalOutput")
        dbg["rec"] = nc.dram_tensor("dbg_rec", [H, SQ], F32, kind="ExternalOutput")
        dbg["praw"] = nc.dram_tensor("dbg_praw", [128, 3 * SQ], F16, kind="ExternalOutput")
        dbg["pr"] = nc.dram_tensor("dbg_pr", [128, 3 * SQ], F16, kind="ExternalOutput")

    # collective bounce buffer (always Internal DRAM); K and V shards ride
    # ONE AllGather — each collective costs ~40us wall (ncfw control latency
    # dominates over bytes), so paying it once is cheaper than twice.
    kvag_in = nc.dram_tensor("kvag_in", [128, KPART + VPART], F16)
    kvag_out = nc.dram_tensor(
        "kvag_out", [NCORES, 128, KPART + VPART], F16, addr_space="Shared"
    )

    with tile.TileContext(nc) as tc:
        _build_tile(tc, qT, kTl, vTl, maskT, w, b, outT, kvag_in, kvag_out, variant, dbg)
    nc.compile()
    return nc


def _build_tile(tc, qT, kTl, vTl, maskT, w, b, outT, kvag_in, kvag_out,
                variant="full", dbg=None):
    nc = tc.nc

    with (
        tc.tile_pool(name="persist", bufs=1) as persist,
        tc.tile_pool(name="wpool", bufs=1) as wpool,
        tc.tile_pool(name="probs", bufs=3) as probs_pool,
        tc.tile_pool(name="rep", bufs=2) as rep_pool,
        tc.tile_pool(name="small", bufs=1) as small,
    ):
        # warm the ACT exp table set during startup (the PSEUDO table load
        # walrus inserts before the first Exp costs ~2.7us; pay it at t=0)
        warm = persist.tile([1, 8], F32, name="warm", tag="warm")
        nc.vector.memset(warm[:], 0.0)
        nc.scalar.activation(
            out=warm[:], in_=warm[:],
            func=mybir.ActivationFunctionType.Exp, bias=0.0, scale=1.0,
        )

        # ---- persistent SBUF tensors ----
        maskT_sb = persist.tile([128, KT_TILES, SQ], F16)
        # scalar engine is idle until attention: its queue hosts the mask DMA
        nc.scalar.dma_start(out=maskT_sb[:], in_=maskT[:])

        # KT_sb chunk-major so each gathered rank slice lands contiguously
        KT_sb = persist.tile([128, NCORES, DT, KSH], F16)
        V_sb = persist.tile([128, KT_TILES, H, DK + 1], F16)
        QT_sb = persist.tile([128, DT, SQ], F16)
        ctx_sb = persist.tile([128, DT, SQ], F16)

        bias_sb = {}
        for x in "qkvo":
            bias_sb[x] = persist.tile([128, DT], F32, name=f"bias_{x}", tag=f"bias_{x}")
            nc.sync.dma_start(out=bias_sb[x][:], in_=b[x][:])
        # bv replicated across partitions (host-prepared) for the V projection
        bv_rep = persist.tile([128, H, DK], F32)
        nc.sync.dma_start(out=bv_rep[:], in_=b["v_rep"][:])

        with (
            tc.tile_pool(name="stage", bufs=2) as stage,
            tc.tile_pool(name="shard", bufs=1) as shard,
            tc.tile_pool(name="pproj", bufs=4, space="PSUM") as pproj,
        ):
            # ---- K shard projection (exchange input ready earliest) ----
            wk_sb = wpool.tile([128, DT, D], F16, tag="w")
            nc.sync.dma_start(out=wk_sb[:], in_=w["k"][:])
            xk = stage.tile([128, DT, KSH], F16, tag="xT")
            nc.sync.dma_start(out=xk[:], in_=kTl[:])
            kshard = shard.tile([128, DT, KSH], F16, tag="ksh")
            for d in range(DT):
                ps = pproj.tile([128, KSH], F32, tag="pj")
                for ka in range(DT):
                    nc.tensor.matmul(
                        ps[:],
                        wk_sb[:, ka, d * 128 : (d + 1) * 128],
                        xk[:, ka, :],
                        start=(ka == 0),
                        stop=(ka == DT - 1),
                    )
                nc.vector.tensor_scalar_add(
                    out=kshard[:, d, :],
                    in0=ps[:],
                    scalar1=bias_sb["k"][:, d : d + 1],
                )
            nc.scalar.dma_start(
                out=kvag_in[:, 0:KPART],
                in_=kshard[:].rearrange("p a b -> p (a b)"),
            )

            # ---- V shard projection ----
            wv_sb = wpool.tile([128, DT, D], F16, tag="w")
            nc.sync.dma_start(out=wv_sb[:], in_=w["v"][:])
            xv = stage.tile([128, DT, KSH], F16, tag="xT")
            nc.sync.dma_start(out=xv[:], in_=vTl[:])
            vshard = shard.tile([128, 4, H, DK + 1], F16, tag="vsh")
            nc.vector.memset(vshard[:, :, :, DK : DK + 1], 1.0)
            for rt in range(4):  # 4 row-tiles of 128 in the 512-row shard
                for half in range(2):  # 2 x 384 output columns
                    ps = pproj.tile([128, 384], F32, tag="pv")
                    for ka in range(DT):
                        nc.tensor.matmul(
                            ps[:],
                            xv[:, ka, rt * 128 : (rt + 1) * 128],
                            wv_sb[:, ka, half * 384 : (half + 1) * 384],
                            start=(ka == 0),
                            stop=(ka == DT - 1),
                        )
                    nc.vector.tensor_add(
                        out=vshard[:, rt, half * 6 : (half + 1) * 6, 0:DK],
                        in0=ps[:].rearrange("p (h e) -> p h e", e=DK),
                        in1=bv_rep[:, half * 6 : (half + 1) * 6, :],
                    )
            nc.scalar.dma_start(
                out=kvag_in[:, KPART : KPART + VPART],
                in_=vshard[:].rearrange("p a h e -> p (a h e)"),
            )
            nc.gpsimd.collective_compute(
                "AllGather",
                mybir.AluOpType.bypass,
                replica_groups=[list(range(NCORES))],
                ins=[kvag_in[:].opt()],
                outs=[kvag_out[:].opt()],
            )
            for c in range(NCORES):
                # V slices on the gpsimd queue (own trigger; needed shortly
                # after attention starts, rank by rank)
                nc.gpsimd.dma_start(
                    out=V_sb[:, 4 * c : 4 * c + 4, :, :],
                    in_=kvag_out[c, :, KPART : KPART + VPART].rearrange(
                        "p (a h e) -> p a h e", h=H, e=DK + 1
                    ),
                )
            for c in range(NCORES):
                # K slices ride the scalar queue: the EXPs queued behind them
                # need K-derived scores anyway, and no other queue is blocked
                nc.scalar.dma_start(
                    out=KT_sb[:, c, :, :],
                    in_=kvag_out[c, :, 0:KPART].rearrange("p (a b) -> p a b", b=KSH),
                )

            # ---- Q projection (overlaps the AllGather) ----
            wq_sb = wpool.tile([128, DT, D], F16, tag="w")
            nc.sync.dma_start(out=wq_sb[:], in_=w["q"][:])
            xq = stage.tile([128, DT, SQ], F16, tag="xT")
            nc.sync.dma_start(out=xq[:], in_=qT[:])
            for d in range(DT):
                ps = pproj.tile([128, SQ], F32, tag="pj")
                for ka in range(DT):
                    nc.tensor.matmul(
                        ps[:],
                        wq_sb[:, ka, d * 128 : (d + 1) * 128],
                        xq[:, ka, :],
                        start=(ka == 0),
                        stop=(ka == DT - 1),
                    )
                nc.vector.tensor_scalar_add(
                    out=QT_sb[:, d, :],
                    in0=ps[:],
                    scalar1=bias_sb["q"][:, d : d + 1],
                )

        # ---- attention, head by head ----
        with tc.tile_pool(name="pattn", bufs=2, space="PSUM") as pattn:
            for h in range(H):
                dt_h = h // 2
                po = 64 * (h % 2)
                lhs_q = QT_sb[po : po + 64, dt_h, :]
                ctx_ps = pattn.tile([128, SQ], F32, tag="ctx")

                def emit_pv(pv_off, pv_g, pv_pr):
                    for j in range(pv_g):
                        kt = pv_off + j
                        nc.tensor.matmul(
                            ctx_ps[0 : DK + 1, :],
                            V_sb[:, kt, h, :],
                            pv_pr[:, j, :],
                            start=(kt == 0),
                            stop=(kt == KT_TILES - 1),
                            skip_group_check=True,
                        )

                # PE stream is software-pipelined one group ahead: scores of
                # group i+1 are issued before probs@V of group i, so the
                # in-order PE queue never blocks on exp/mask of group i.
                pending = None
                off = 0
                for gi, g in enumerate(EXP_GROUPS):
                    sc = pattn.tile([128, 3, SQ], F32, tag="sc3")
                    for j in range(g):
                        kt = off + j
                        nc.tensor.matmul(
                            sc[:, j, :],
                            KT_sb[po : po + 64, kt // 4, dt_h,
                                  (kt % 4) * 128 : (kt % 4 + 1) * 128],
                            lhs_q,
                            start=True,
                            stop=True,
                        )
                    if pending is not None:
                        emit_pv(*pending)
                    # exp straight from PSUM on ACT; 1/sqrt(dk) folded into
                    # the activation affine. Mask applied after, as a {0,1}
                    # fp16 multiplicand, at DVE 2x rate.
                    pr_raw = probs_pool.tile([128, 3, SQ], F16, tag="praw")
                    nc.scalar.activation(
                        out=pr_raw[:, 0:g, :],
                        in_=sc[:, 0:g, :],
                        func=mybir.ActivationFunctionType.Exp,
                        bias=0.0,
                        scale=float(1.0 / np.sqrt(DK)),
                    )
                    pr = probs_pool.tile([128, 3, SQ], F16, tag="pr")
                    mul_eng = nc.gpsimd if gi in GPSIMD_GROUPS else nc.vector
                    mul_eng.tensor_mul(
                        out=pr[:, 0:g, :],
                        in0=pr_raw[:, 0:g, :],
                        in1=maskT_sb[:, off : off + g, :],
                    )
                    if variant == "debug" and h == 0 and off == 0:
                        nc.sync.dma_start(
                            out=dbg["praw"][:],
                            in_=pr_raw[:].rearrange("p a b -> p (a b)"),
                        )
                        nc.sync.dma_start(
                            out=dbg["pr"][:], in_=pr[:].rearrange("p a b -> p (a b)")
                        )
                    pending = (off, g, pr)
                    off += g
                emit_pv(*pending)
                # normalize: rows 0..63 are ctx^T, row 64 is the denominator.
                # reciprocal_approx_fast (~51 ULP) is ~5x cheaper than the
                # exact DVE reciprocal; it misreads PSUM operands, so the
                # denominator row goes through SBUF.
                dent = small.tile([1, SQ], F32, tag="dent")
                nc.vector.tensor_copy(out=dent[:], in_=ctx_ps[DK : DK + 1, :])
                recip = small.tile([1, SQ], F32, tag="recip")
                nc.vector.reciprocal_approx_fast(out=recip[:], in_=dent[:])
                if variant == "debug":
                    nc.sync.dma_start(out=dbg["den"][h : h + 1, :], in_=dent[:])
                    nc.sync.dma_start(out=dbg["rec"][h : h + 1, :], in_=recip[:])
                rep = rep_pool.tile([DK, SQ], F32, tag="rep")
                nc.gpsimd.partition_broadcast(rep[:], recip[:])
                nc.vector.tensor_mul(
                    out=ctx_sb[po : po + 64, dt_h, :],
                    in0=ctx_ps[0:DK, :],
                    in1=rep[:],
                )

        if variant == "debug":
            nc.sync.dma_start(
                out=dbg["kt"][:], in_=KT_sb[:].rearrange("p c a b -> p (c a b)")
            )
            nc.sync.dma_start(
                out=dbg["v"][:], in_=V_sb[:].rearrange("p a h e -> p (a h e)")
            )
            nc.sync.dma_start(
                out=dbg["q"][:], in_=QT_sb[:].rearrange("p a b -> p (a b)")
            )
            nc.sync.dma_start(
                out=dbg["ctx"][:], in_=ctx_sb[:].rearrange("p a b -> p (a b)")
            )

        # ---- output projection ----
        with tc.tile_pool(name="pout", bufs=2, space="PSUM") as pout:
            wo_sb = wpool.tile([128, DT, D], F16, tag="w")
            nc.sync.dma_start(out=wo_sb[:], in_=w["o"][:])
            for d in range(DT):
                ps = pout.tile([128, SQ], F32, tag="po")
                for ka in range(DT):
                    nc.tensor.matmul(
                        ps[:],
                        wo_sb[:, ka, d * 128 : (d + 1) * 128],
                        ctx_sb[:, ka, :],
                        start=(ka == 0),
                        stop=(ka == DT - 1),
                    )
                o_sb = small.tile([128, SQ], F32, tag="osb")
                nc.vector.tensor_scalar_add(
                    out=o_sb[:],
                    in0=ps[:],
                    scalar1=bias_sb["o"][:, d : d + 1],
                )
                nc.sync.dma_start(out=outT[d * 128 : (d + 1) * 128, :], in_=o_sb[:])


def _tile_dm(x):
    """[D, N] -> [128, D//128, N] fp16 (partition-tiled over the first dim)."""
    n = x.shape[1]
    return np.ascontiguousarray(
        x.reshape(DT, 128, n).swapaxes(0, 1).astype(np.float16)
    )


def _prep_inputs(q, k, v, mask, wq, bq, wk, bk, wv, bv, wo, bo):
    q = np.asarray(q, dtype=np.float32).reshape(S, D)
    k = np.asarray(k, dtype=np.float32).reshape(S, D)
    v = np.asarray(v, dtype=np.float32).reshape(S, D)
    mask = np.asarray(mask).reshape(S, S)

    kT_t = _tile_dm(k.T)  # [128, 6, 4096]
    vT_t = _tile_dm(v.T)
    w_t = {
        "q": _tile_dm(np.asarray(wq, np.float32)),
        "k": _tile_dm(np.asarray(wk, np.float32)),
        "v": _tile_dm(np.asarray(wv, np.float32)),
        "o": _tile_dm(np.asarray(wo, np.float32)),
    }
    b_t = {
        "q": np.ascontiguousarray(np.asarray(bq, np.float32).reshape(DT, 128).T),
        "k": np.ascontiguousarray(np.asarray(bk, np.float32).reshape(DT, 128).T),
        "v": np.ascontiguousarray(np.asarray(bv, np.float32).reshape(DT, 128).T),
        "o": np.ascontiguousarray(np.asarray(bo, np.float32).reshape(DT, 128).T),
    }
    bv_rep = np.ascontiguousarray(
        np.broadcast_to(np.asarray(bv, np.float32).reshape(1, H, DK), (128, H, DK))
    )

    in_maps = []
    for c in range(NCORES):
        qs, qe = c * SQ, (c + 1) * SQ
        m = {
            "qT": _tile_dm(q[qs:qe, :].T),
            "kTl": np.ascontiguousarray(kT_t[:, :, c * KSH : (c + 1) * KSH]),
            "vTl": np.ascontiguousarray(vT_t[:, :, c * KSH : (c + 1) * KSH]),
            "maskT": np.ascontiguousarray(
                mask[qs:qe, :].T.reshape(KT_TILES, 128, SQ).swapaxes(0, 1)
            ).astype(np.float16),
        }
        for x in "qkvo":
            m[f"w{x}"] = w_t[x]
            m[f"b{x}"] = b_t[x]
        m["bv_rep"] = bv_rep
        in_maps.append(m)
    return in_maps


def kernel(**inputs) -> np.ndarray:
    if "nc" not in _CACHE:
        _CACHE["nc"] = build_kernel()
    nc = _CACHE["nc"]
    in_maps = _prep_inputs(**inputs)
    res = bass_utils.run_bass_kernel_spmd(nc, in_maps, core_ids=list(range(NCORES)))
    out = np.concatenate(
        [res.results[c]["outT"].T for c in range(NCORES)], axis=0
    ).astype(np.float32)
    return out.reshape(B, S, D)
